# revision 22
# baseline (speedup 1.0000x reference)
"""PrRoIPool (Precise RoI Pooling) Trainium2 Bass kernel.

Problem: features [2, 256, 100, 100] f32, rois [256, 5] f32 ->
out [256, 256, 7, 7] f32 where
  out[n,c,p,q] = (1/area) * sum_{h,w} F[bi,c,h,w] * wy[n,p,h] * wx[n,q,w]
with wy/wx the exact integrals of the bilinear-interp hat functions over
each pooling bin (separable).

Strategy (8 NeuronCores, SPMD):
  - Host: compute hat-integral weights wy [N,7,H], wx [N,7,W] (tiny:
    ~0.004%% of total FLOPs), fold 1/bin_h into wy and 1/bin_w into wx.
  - Shard ROIs by batch image: cores 0-3 take batch-0 ROIs, cores 4-7
    batch-1 (S slots per core, zero-padded). Each core holds the full
    feature image of its batch, pre-transposed to [W, C, H] fp16.
  - Stage A (per channel c): T1[h, c', (s,q)] = F[c].T @ wx -- PE matmul,
    stationary [w=100, h=100], moving [w=100, S*7], fp32 PSUM, then
    cast-copy PSUM->SBUF fp16, rotating over Vector/Scalar/GpSimd by a
    cost-balanced greedy (the copies are the pacing resource).
  - Stage B (per 4-ROI group g, per 64-channel quarter j):
    out[p, (c',q)] = wy_s.T @ T1[:, c', s, :] -- 4 ROIs col-tiled into
    one PSUM bank at partition offsets 0/32/64/96 (concurrent in the PE
    array), PSUM->SBUF fp16 staging, packed partition-strided DMA out
    (only the 7 valid rows per 32-row block leave the chip).
  - Quarter-3 stage B is emitted in 32-channel halves interleaved into
    stage A's last channels so the kernel has no serialized tail.
"""

import sys

if "/opt/trn_rl_repo" not in sys.path:
    sys.path.insert(0, "/opt/trn_rl_repo")

import numpy as np

POOLED = 7
SPATIAL_SCALE = 0.0625
B, C, H, W = 2, 256, 100, 100
N_CORES = 8
CORES_PER_BATCH = 4
S_CAP = 36  # max ROI slots per core (stage-A PSUM slot is half a bank)

_prog_cache = {}


def _hat_cdf(u):
    return np.where(
        u <= 0.0,
        0.5 * np.clip(u + 1.0, 0.0, 1.0) ** 2,
        1.0 - 0.5 * np.clip(1.0 - u, 0.0, 1.0) ** 2,
    )


def _bin_weights(lo, hi, size):
    # [N, P] bounds -> [N, P, size] integral of hat centered at each index
    idx = np.arange(size, dtype=lo.dtype)
    return _hat_cdf(hi[..., None] - idx) - _hat_cdf(lo[..., None] - idx)


def _host_weights(rois):
    """Per-ROI separable weights with 1/area folded in. float32."""
    r = rois.astype(np.float64)
    x1 = r[:, 1] * SPATIAL_SCALE
    y1 = r[:, 2] * SPATIAL_SCALE
    x2 = r[:, 3] * SPATIAL_SCALE
    y2 = r[:, 4] * SPATIAL_SCALE
    bw = (x2 - x1) / POOLED
    bh = (y2 - y1) / POOLED
    ph = np.arange(POOLED, dtype=np.float64)
    ylo = y1[:, None] + ph * bh[:, None]
    yhi = ylo + bh[:, None]
    xlo = x1[:, None] + ph * bw[:, None]
    xhi = xlo + bw[:, None]
    wy = _bin_weights(ylo, yhi, H)  # [N, 7, H]
    wx = _bin_weights(xlo, xhi, W)  # [N, 7, W]
    # reference: out = einsum / max(area,1e-12) where area = bw*bh, zeroed
    # if area <= 0. Fold 1/bh into wy, 1/bw into wx (area > 0 case).
    ok = (bw * bh) > 0.0
    inv_bh = np.where(ok, 1.0 / np.maximum(bh, 1e-12), 0.0)
    inv_bw = np.where(ok, 1.0 / np.maximum(bw, 1e-12), 0.0)
    wy = wy * inv_bh[:, None, None]
    wx = wx * inv_bw[:, None, None]
    return wy.astype(np.float32), wx.astype(np.float32)


def _build_program(S):
    """Bass/Tile SPMD program for S ROI slots per core. Cached per S."""
    from contextlib import ExitStack

    from concourse import bacc, mybir
    import concourse.tile as tile

    f16 = mybir.dt.float16
    f32 = mybir.dt.float32
    SQ = S * POOLED
    assert SQ <= 256  # stage-A psum: 2 channel slots of 256 f32 = 1 bank
    NG = -(-S // 4)  # 4-ROI stage-B groups (col-tiled 0/32/64/96)

    nc = bacc.Bacc("TRN2", target_bir_lowering=False, debug=False,
                   num_devices=N_CORES)
    fwt = nc.dram_tensor("fwt", [W, C, H], f16, kind="ExternalInput")
    wxt = nc.dram_tensor("wxt", [W, SQ], f16, kind="ExternalInput")
    # wyt is padded to 32 cols per slot (25 zero) so each stage-B matmul
    # writes a full 32-partition PSUM block (cost is N-driven, M is free).
    wyt = nc.dram_tensor("wyt", [H, S * 32], f16, kind="ExternalInput")
    # packed output: per (half, roi-in-group): 7 valid rows of every group
    out = nc.dram_tensor("out", [2, 4, POOLED, 2, NG, 448], f16,
                         kind="ExternalOutput")

    # feature-chunk schedule: small chunks first for a fast pipeline start
    chunks = [2, 2, 4, 8, 16, 16, 16] + [32] * 6
    assert sum(chunks) == C
    starts = np.cumsum([0] + chunks).tolist()

    with tile.TileContext(nc) as tc, ExitStack() as ctx:
        sb = ctx.enter_context(tc.tile_pool(name="sb", bufs=1))
        fw_pool = ctx.enter_context(tc.tile_pool(name="fw", bufs=4))
        t1_pool = ctx.enter_context(tc.tile_pool(name="t1", bufs=3))
        pa_pool = ctx.enter_context(tc.tile_pool(name="pa", bufs=3,
                                                 space="PSUM"))
        pb_pool = ctx.enter_context(tc.tile_pool(name="pb", bufs=2,
                                                 space="PSUM"))

        wx_t = sb.tile([W, SQ], f16, tag="wx")
        nc.sync.dma_start(out=wx_t[:], in_=wxt[:])
        wy_t = sb.tile([H, S * 32], f16, tag="wy")
        nc.sync.dma_start(out=wy_t[:], in_=wyt[:])

        # PE warmup: dense matmuls while the first feature DMAs land, to
        # flip the HAM clock gate to 8/8 (PE runs at 1.2 GHz until it sees
        # a ~3.4us window of sustained activity; everything after runs 2x)
        warm = sb.tile([128, 640], f16, tag="warm")
        nc.gpsimd.memset(warm[:], 0.0)
        wps = pb_pool.tile([128, 512], f32, tag="pb")
        for _ in range(6):
            nc.tensor.matmul(wps[:, 0:512], lhsT=warm[:, 0:128],
                             rhs=warm[:, 128:640])

        def keepalive():
            ka = pb_pool.tile([128, 512], f32, tag="pb")
            nc.tensor.matmul(ka[:, 0:512], lhsT=warm[:, 0:128],
                             rhs=warm[:, 128:640])

        # strictly alternating copy dispatch over Vector/Scalar (the
        # only PSUM readers): consecutive copies never stack on one engine
        n_copy = [0]

        def emit_copy(dst, src, nelem, stage_a=False):
            if n_copy[0] % 2 == 0:
                nc.vector.tensor_copy(dst, src)
            else:
                nc.scalar.copy(dst, src)
            n_copy[0] += 1

        # single persistent staging tile for all groups: [128, NG, 4, 448].
        # copies land at partition 32*i for ROI i-of-group; the 7 valid
        # rows per 32-block leave via 8 total packed DMAs (4 per half).
        stg = sb.tile([128, 4, NG, 448], f16, tag="stg")

        t1b = {}  # quarter j -> t1 tile [H, 64, S, 7] (c-major)

        def emit_b(j, g, block=None):
            # stage-B for quarter j, 4-ROI group g, col-tiled at PSUM
            # offsets 0/32/64/96. block None: all 64 channels of the
            # quarter; block 0/1: a 32-channel half, emitted as a PAIR of
            # groups (g, g+1) sharing one PSUM bank + one copy (tail
            # pipelining with half the chain length).
            if block is None:
                gs = [g]
                c0, c1 = 0, 64
            else:
                gs = [gg for gg in (g, g + 1) if gg < NG]
                c0, c1 = 32 * block, 32 * block + 32
            x0, x1 = c0 * POOLED, c1 * POOLED
            xw = x1 - x0
            pb = pb_pool.tile([128, 512], f32, tag="pb")
            hi = 0
            for gi, gg in enumerate(gs):
                rois_g = [s for s in range(4 * gg, min(4 * gg + 4, S))]
                hi = max(hi, 32 * len(rois_g))
                for i, s in enumerate(rois_g):
                    nc.tensor.matmul(
                        pb[32 * i:32 * i + 32, gi * xw:gi * xw + xw],
                        lhsT=wy_t[:, s * 32:(s + 1) * 32],
                        rhs=t1b[j][:, c0:c1, s * POOLED:(s + 1) * POOLED],
                        tile_position=(0, 32 * i),
                    )
            emit_copy(stg[0:hi, j, gs[0]:gs[0] + len(gs), x0:x1],
                      pb[0:hi, 0:len(gs) * xw], len(gs) * xw)

        def emit_out_dma(hf, glo=0, ghi=None):
            # packed DMAs (one per ROI-in-group): contiguous 7-partition
            # slices; (j, g, x) inner dims stay contiguous per partition
            # row, so descriptors stay big and the drain is fast
            ghi = NG if ghi is None else ghi
            for i in range(4):
                eng = nc.gpsimd if i % 2 == 0 else nc.sync
                eng.dma_start(
                    out=out[hf, i, :, :, glo:ghi],
                    in_=stg[32 * i:32 * i + POOLED,
                            2 * hf:2 * hf + 2, glo:ghi, :])

        fw_tiles = {}

        def issue_chunk(k):
            if k >= len(chunks) or k in fw_tiles:
                return
            t = fw_pool.tile([W, 32, H], f16, tag="fw")
            nc.sync.dma_start(out=t[:, 0:chunks[k], :],
                              in_=fwt[:, starts[k]:starts[k] + chunks[k], :])
            fw_tiles[k] = t

        for k in range(3):
            issue_chunk(k)
        chunk_idx = 0
        cur = None
        coff = 0
        pa = None
        for j in range(4):
            # ---- Stage A quarter: T1_j[h, c', s, q] = F[c].T @ wx ----
            # with the previous quarter's stage-B groups interleaved to
            # fill the copy-paced gaps on the PE; quarter 3 additionally
            # interleaves its own first-half stage-B (tail removal)
            t1b[j] = t1_pool.tile([H, 64, SQ], f16, tag="t1",
                                  name=f"t1_{j}")
            qa = [(j - 1, g, None) for g in range(NG)] if j > 0 else []
            qb = ([(3, g, 0) for g in range(0, NG, 2)]
                  if j == 3 else [])
            last_extra = -1
            for ci, c in enumerate(range(64 * j, 64 * (j + 1))):
                if chunk_idx < len(chunks) and c == starts[chunk_idx]:
                    if chunks[chunk_idx] >= 32:
                        # pad the possible chunk-DMA wait so the HAM clock
                        # gate sees a busy PE (in-order queue: pads must
                        # precede the matmul that would stall)
                        keepalive()
                    cur = fw_tiles.pop(chunk_idx)
                    issue_chunk(chunk_idx + 3)
                    coff = c
                    chunk_idx += 1
                if ci % 4 == 0:
                    pa = pa_pool.tile([H, 4, 256], f32, tag="pa")
                nc.tensor.matmul(
                    pa[:, ci % 4, 0:SQ],
                    lhsT=cur[:, c - coff, :],
                    rhs=wx_t[:],
                )
                if ci % 4 == 3:
                    emit_copy(t1b[j][:, ci - 3:ci + 1, :],
                              pa[:, :, 0:SQ], 4 * SQ, stage_a=True)
                if j < 3:
                    if ci % 5 == 4 and qa:
                        emit_b(*qa.pop(0))
                        last_extra = ci
                    elif ci - last_extra >= 12:
                        # PE idle gaps while copy/DMA-paced re-throttle the
                        # HAM clock gate; keepalives hold 8/8 (baseline
                        # lesson: the HAM threshold is unforgiving)
                        keepalive()
                        last_extra = ci
                else:
                    if ci % 3 == 1 and (qa or (ci >= 33 and qb)):
                        emit_b(*(qa.pop(0) if qa else qb.pop(0)))
                        last_extra = ci
                    elif ci - last_extra >= 12:
                        keepalive()
                        last_extra = ci
            for item in qa:
                emit_b(*item)
            for item in qb:
                emit_b(*item)
            keepalive()
            if j == 2:
                emit_out_dma(0)
        # tail: second halves of quarter-3 stage B (first halves already
        # interleaved above), with keepalives between pairs
        gsplit = 2 * ((NG // 2 + 1) // 2)  # after half the pairs
        for g in range(0, NG, 2):
            emit_b(3, g, 1)
            if g + 2 == gsplit:
                emit_out_dma(1, 0, gsplit)
            if (g // 2) % 2 == 1:
                keepalive()
        emit_out_dma(1, gsplit, NG)

    nc.compile()
    return nc


def _plan_shards(bi, n_rois):
    """Assign ROI indices to (wave, core, slot). Returns S and a list of
    per-wave assignment arrays of shape [N_CORES, S] (-1 = padding)."""
    groups = [np.where(bi == b)[0] for b in range(B)]
    need = max((len(g) + CORES_PER_BATCH - 1) // CORES_PER_BATCH
               for g in groups)
    need = max(need, 1)
    S = min(need, S_CAP)
    per_wave_cap = S * CORES_PER_BATCH
    n_waves = max(-(-len(g) // per_wave_cap) for g in groups)
    waves = []
    for wv in range(n_waves):
        asg = np.full((N_CORES, S), -1, dtype=np.int64)
        for b in range(B):
            g = groups[b][wv * per_wave_cap:(wv + 1) * per_wave_cap]
            for k in range(CORES_PER_BATCH):
                chunk = g[k * S:(k + 1) * S]
                asg[b * CORES_PER_BATCH + k, :len(chunk)] = chunk
        waves.append(asg)
    return S, waves


def kernel(features, rois, _trace=False):
    from concourse.bass_utils import run_bass_kernel_spmd

    features = np.asarray(features, dtype=np.float32)
    rois = np.asarray(rois, dtype=np.float32)
    n_rois = rois.shape[0]
    bi = np.rint(rois[:, 0]).astype(np.int64)
    bi = np.where((bi >= 0) & (bi < B), bi, -1)

    wy, wx = _host_weights(rois)  # [N, 7, H] / [N, 7, W], 1/area folded
    S, waves = _plan_shards(bi, n_rois)

    if S not in _prog_cache:
        _prog_cache[S] = _build_program(S)
    nc = _prog_cache[S]

    # Features per batch, transposed to [W, C, H], fp16. Shared across the
    # 4 cores of each batch group.
    fwt = [np.ascontiguousarray(features[b].transpose(2, 0, 1))
           .astype(np.float16) for b in range(B)]

    out_full = np.zeros((n_rois, C, POOLED, POOLED), dtype=np.float32)
    exec_ns = None
    for asg in waves:
        in_maps = []
        for k in range(N_CORES):
            wxt = np.zeros((W, S * POOLED), dtype=np.float16)
            wyt = np.zeros((H, S * 32), dtype=np.float16)
            for s in range(S):
                r = asg[k, s]
                if r < 0:
                    continue
                # w[n, q, w-axis] -> [w-axis, s*7+q]
                wxt[:, s * POOLED:(s + 1) * POOLED] = \
                    wx[r].T.astype(np.float16)
                wyt[:, s * 32:s * 32 + POOLED] = \
                    wy[r].T.astype(np.float16)
            in_maps.append({
                "fwt": fwt[k // CORES_PER_BATCH],
                "wxt": wxt,
                "wyt": wyt,
            })
        res = run_bass_kernel_spmd(nc, in_maps, list(range(N_CORES)),
                                   trace=_trace)
        if res.exec_time_ns is not None:
            exec_ns = max(exec_ns or 0, res.exec_time_ns)
        for k in range(N_CORES):
            arr = res.results[k]["out"]  # [2, 4, 7, 2, NG, 448] f16
            for s in range(S):
                r = asg[k, s]
                if r < 0:
                    continue
                g, i = divmod(s, 4)
                # [hf, p, jj, c', q] -> [hf, jj, c', p, q] -> [C, 7, 7]
                blk = (arr[:, i, :, :, g]
                       .reshape(2, POOLED, 2, 64, POOLED)
                       .transpose(0, 2, 3, 1, 4)
                       .reshape(C, POOLED, POOLED))
                out_full[r] = blk.astype(np.float32)

    if _trace:
        kernel.last_exec_time_ns = exec_ns
    return out_full
